# revision 18
# baseline (speedup 1.0000x reference)
"""Trainium2 Bass kernel for batched cross-attention (CoupletsAttentionModel).

Reference computation (per batch element b):
    S = dec @ enc^T          [S_dec, S_enc]
    P = softmax(S, axis=-1)
    O = P @ enc              [S_dec, D]

Sharding: data-parallel over batch — B=8 batch elements, one per NeuronCore.
Each core runs an identical (SPMD) program on its own batch slice; no
collectives, host stacks the 8 per-core outputs.

Per-core algorithm (S_enc=S_dec=2048, D=512, fp32 in/out), v4:
  - fp16 matmuls (4x faster than fp32; out rel err ~2e-3, tolerance 2e-2).
  - Streamed prologue: dec tiles 0-2 (gpsimd queue) + enc tiles (sync/scalar
    queues, alternating) load while casts (f32->f16), PE transposes
    (enc^T/dec^T), and q-tile 0's S matmuls run incrementally per arriving
    enc chunk — prep is DMA-bound instead of serialized before compute.
  - Software-pipelined main loop, per q-tile iteration:
      mm2(qt-1)  [PE ready immediately — its P^T transpose ran last iter]
      mm1(qt+1)  [fills score PSUM banks as exp(qt) releases them]
      softmax(qt): 4x chunk max (DVE) -> global max -> single exp pass
                   (ScalarE, accum row-sums) — no chunk-rescale multiplies
      P^T via one xbar transpose-DMA [128,2048]->[128,16,128] on SP queue
      dec tile qt+3 prep (gpsimd DMA + cast + PE transpose)
  - PSUM budget: 5 score banks [128,512]f32 + 2 out banks + 1 transpose = 8.
  - Engine split: SP = transposes + half enc loads; Scalar = exp + casts +
    out stores + half enc loads; GpSimd = dec loads; DVE = reductions/scales.
"""

import contextlib
import ctypes
import os
import sys
import types

import numpy as np

import concourse.bass as bass
import concourse.tile as tile
from concourse import bacc, mybir
from concourse import bass_utils
from concourse.masks import make_identity

F32 = mybir.dt.float32
F16 = mybir.dt.float16
AX = mybir.AxisListType
AFT = mybir.ActivationFunctionType

N_CORES = 8
PART = 128


def attention_tile_kernel(tc, out_ap, dec_ap, enc_ap, seq, d):
    nc = tc.nc
    P = PART
    KC = 512  # score chunk width = one fp32 PSUM bank
    n_qt = seq // P
    n_kt = seq // P
    n_dt = d // P
    n_ch = seq // KC
    kt_per_ch = KC // P

    stack = contextlib.ExitStack()
    pool = lambda **kw: stack.enter_context(tc.tile_pool(**kw))

    singles = pool(name="singles", bufs=1)
    big = pool(name="big", bufs=1)
    stage = pool(name="stage", bufs=3)
    stage16 = pool(name="stage16", bufs=3)
    encg_pool = pool(name="encg", bufs=4)
    psum = pool(name="psum", bufs=1, space="PSUM")
    p_pool = pool(name="p_pool", bufs=3)
    pt_pool = pool(name="pt_pool", bufs=2)
    stats = pool(name="stats", bufs=4)
    osb = pool(name="osb", bufs=2)

    with stack:
        # dec tile 0 DMA first on the scalar queue so its data is in flight
        # during engine startup (gpsimd is busy with make_identity).
        d32_0 = stage.tile([P, d], F32, tag="ld32", name="d32_0")
        nc.scalar.dma_start(out=d32_0[:], in_=dec_ap[0:P, :])

        ident = singles.tile([P, P], F16)
        make_identity(nc, ident[:])

        d32_12 = singles.tile([P, 2, d], F32, name="d32_12")
        nc.scalar.dma_start(
            out=d32_12[:],
            in_=dec_ap[P : 3 * P, :].rearrange("(i p) c -> p i c", p=P),
        )

        v_sb = big.tile([P, n_kt, d], F16)  # enc natural (V)
        kT = big.tile([P, n_dt, seq], F16)  # enc^T  [d_in, dt, k]
        qT = big.tile([P, n_dt, seq], F16)  # dec^T  [d_in, dt, q]

        # enc as 4 grouped DMAs (one trigger per 1MB: per-tile triggers leave
        # ~1us of dead time between 0.7us transfers), alternating sync/scalar
        # trigger queues so descriptor generation overlaps transfer.
        eb = []
        for g in range(n_ch):
            ebg = encg_pool.tile([P, kt_per_ch, d], F32, tag="encg", name=f"eb_{g}")
            eng = nc.sync if g % 2 == 0 else nc.scalar
            eng.dma_start(
                out=ebg[:],
                in_=enc_ap[g * KC : (g + 1) * KC, :].rearrange(
                    "(i p) c -> p i c", p=P
                ),
            )
            eb.append(ebg)

        # HAM warmup: dense burst of dummy matmuls during the DMA wait keeps
        # the PE clock ramping (2.4 GHz needs ~3us of continuous activity;
        # idle drops it to 1.2 GHz). Sourced from ident; output never read.
        warm_ps = psum.tile([P, d], F32, tag="tps", bufs=2, name="warm_ps")
        for i in range(44):
            nc.tensor.matmul(
                warm_ps[:, :P], ident[:], ident[:], start=(i == 0), stop=(i == 43)
            )

        def transpose4(dst4, src16, base):
            # 4 PE transposes into one PSUM bank, one [128, 4, 128] copy out
            tps = psum.tile(
                [P, n_dt, P], F16, tag="tps", bufs=2,
                name=f"tps_{dst4.tensor.name}_{base}",
            )
            for j in range(n_dt):
                nc.tensor.transpose(
                    tps[:, j, :], src16[:, (base + j) * P : (base + j + 1) * P],
                    ident[:],
                )
            nc.vector.tensor_copy(dst4[:], tps[:])

        def prep_transposed(src16, dstT, st):
            transpose4(dstT[:, :, st * P : (st + 1) * P], src16, 0)

        def prep_dec_from(src32, j):
            d16 = stage16.tile([P, d], F16, tag="d16", name=f"d16_{j}")
            nc.vector.tensor_copy(d16[:], src32)
            prep_transposed(d16[:], qT, j)

        prev_transpose = [None]

        def prep_dec(j):
            # main-loop dec prep rides the (otherwise idle) xbar: one
            # transpose-DMA replaces 4 PE transposes + a vector copy. One
            # chain link per iteration, consumed 2 iterations later.
            d32 = stage.tile([P, d], F32, tag="ld32", name=f"d32_{j}")
            nc.sync.dma_start(out=d32[:], in_=dec_ap[j * P : (j + 1) * P, :])
            d16 = stage16.tile([P, d], F16, tag="d16", name=f"d16_{j}")
            nc.vector.tensor_copy(d16[:], d32[:])
            ti = nc.sync.dma_start(
                out=qT[:, :, j * P : (j + 1) * P], in_=d16[:], transpose=True
            )
            if prev_transpose[0] is not None:
                tile.add_dep_helper(
                    ti.ins, prev_transpose[0].ins, reason="serialize xbar"
                )
            prev_transpose[0] = ti

        # ---- phase A: stream enc groups; build kT/v_sb/qT0-2; qt0's S ----
        prep_dec_from(d32_0[:], 0)
        s_cur = [
            psum.tile([P, KC], F32, tag="s_ch", bufs=5, name=f"s_0_{c}")
            for c in range(n_ch)
        ]
        nmx_cur = stats.tile([P, n_ch], F32, tag="nmx", name="nmx_0")
        for g in range(n_ch):
            for j in range(kt_per_ch):
                st = g * kt_per_ch + j
                nc.scalar.activation(v_sb[:, st, :], eb[g][:, j, :], AFT.Copy)
                prep_transposed(v_sb[:, st, :], kT, st)
            for dt in range(n_dt):
                nc.tensor.matmul(
                    s_cur[g][:],
                    qT[:, dt, 0:P],
                    kT[:, dt, g * KC : (g + 1) * KC],
                    start=(dt == 0),
                    stop=(dt == n_dt - 1),
                )
            nc.vector.tensor_reduce(
                nmx_cur[:, g : g + 1], s_cur[g][:], axis=AX.X,
                op=mybir.AluOpType.max, negate=True,
            )
            if g < 2:
                prep_dec_from(d32_12[:, g, :], g + 1)

        pending = None  # (pT3, rinv) of tile awaiting mm2

        def mm2(qt, pT3, rinv):
            o_ch = psum.tile([P, d], F32, tag="o_ch", bufs=1, name=f"o_ch_{qt}")
            for kt in range(n_kt):
                nc.tensor.matmul(
                    o_ch[:],
                    pT3[:, kt, :],
                    v_sb[:, kt, :],
                    start=(kt == 0),
                    stop=(kt == n_kt - 1),
                )
            o_sb = osb.tile([P, d], F32, tag="osb", name=f"o_sb_{qt}")
            nc.vector.tensor_scalar_mul(o_sb[:], o_ch[:], rinv[:])
            nc.sync.dma_start(out=out_ap[qt * P : (qt + 1) * P, :], in_=o_sb[:])

        # ---- software-pipelined main loop ----
        for qt in range(n_qt):
            # softmax(qt): global max -> exp chunks (ScalarE, row-sum accum)
            negmg = stats.tile([P, 1], F32, tag="negmg", name=f"negmg_{qt}")
            nc.vector.tensor_reduce(
                negmg[:], nmx_cur[:], axis=AX.X, op=mybir.AluOpType.min
            )
            p_sb = p_pool.tile([P, seq], F16, tag="p", name=f"p_{qt}")
            sums = stats.tile([P, n_ch], F32, tag="sums", name=f"sums_{qt}")
            for c in range(n_ch):
                nc.scalar.activation(
                    p_sb[:, c * KC : (c + 1) * KC],
                    s_cur[c][:],
                    AFT.Exp,
                    bias=negmg[:],
                    scale=1.0,
                    accum_out=sums[:, c : c + 1],
                )
            sm = stats.tile([P, 1], F32, tag="sm", name=f"sm_{qt}")
            nc.vector.reduce_sum(sm[:], sums[:], axis=AX.X)
            rinv = stats.tile([P, 1], F32, tag="rinv", name=f"rinv_{qt}")
            nc.vector.reciprocal(rinv[:], sm[:])

            # mm2 of the previous tile first: the PE starts immediately on it
            # while exp(qt) runs and frees score banks.
            if pending is not None:
                mm2(qt - 1, *pending)

            # P^T on the PE (16 transposes, ~0.9us): the xbar transpose-DMA
            # takes ~6.3us serialized (1.7us fixed trigger + 4.5us transfer)
            # which exceeds the 6.8us PE iteration and stalls mm2 every tile.
            pT3 = pt_pool.tile([P, n_kt, P], F16, tag="pT", name=f"pT_{qt}")
            for c in range(n_ch):
                transpose4(
                    pT3[:, c * kt_per_ch : (c + 1) * kt_per_ch, :],
                    p_sb, c * kt_per_ch,
                )
            pending = (pT3, rinv)

            # mm1(qt+1), each chunk's row-max reduced right behind it
            if qt + 1 < n_qt:
                q0 = (qt + 1) * P
                s_nxt = [
                    psum.tile([P, KC], F32, tag="s_ch", bufs=5, name=f"s_{qt+1}_{c}")
                    for c in range(n_ch)
                ]
                nmx_nxt = stats.tile([P, n_ch], F32, tag="nmx", name=f"nmx_{qt+1}")
                for c in range(n_ch):
                    for dt in range(n_dt):
                        nc.tensor.matmul(
                            s_nxt[c][:],
                            qT[:, dt, q0 : q0 + P],
                            kT[:, dt, c * KC : (c + 1) * KC],
                            start=(dt == 0),
                            stop=(dt == n_dt - 1),
                        )
                    nc.vector.tensor_reduce(
                        nmx_nxt[:, c : c + 1], s_nxt[c][:], axis=AX.X,
                        op=mybir.AluOpType.max, negate=True,
                    )
                s_cur = s_nxt
                nmx_cur = nmx_nxt

            if qt + 3 < n_qt:
                prep_dec(qt + 3)

        mm2(n_qt - 1, *pending)


def build(seq=2048, d=512, n_cores=N_CORES):
    nc = bacc.Bacc(
        "TRN2", target_bir_lowering=False, debug=False, num_devices=n_cores
    )
    dec = nc.dram_tensor("dec", [seq, d], F32, kind="ExternalInput").ap()
    enc = nc.dram_tensor("enc", [seq, d], F32, kind="ExternalInput").ap()
    out = nc.dram_tensor("out", [seq, d], F32, kind="ExternalOutput").ap()
    with tile.TileContext(nc) as tc:
        attention_tile_kernel(tc, out, dec, enc, seq, d)
    nc.compile()
    return nc


# ---------------------------------------------------------------------------
# Optional NTFF profiling support (used by our own test harness; inert unless
# BASSKERNEL_TRACE=1). The agent image lacks `antenv.axon_hooks`, so recreate
# it in sys.modules with a ctypes hook against libaxon_pjrt.so.
# ---------------------------------------------------------------------------
LAST_EXEC_TIME_NS = None


def _install_profile_hook():
    so_path = "/opt/axon/libaxon_pjrt.so"
    if "antenv.axon_hooks" in sys.modules or not os.path.exists(so_path):
        return
    lib = ctypes.CDLL(so_path)
    if not hasattr(lib, "axon_start_nrt_profile"):
        return
    lib.axon_start_nrt_profile.argtypes = [
        ctypes.POINTER(ctypes.c_int64),
        ctypes.c_size_t,
    ]
    lib.axon_start_nrt_profile.restype = ctypes.c_int64
    lib.axon_stop_nrt_profile.argtypes = [ctypes.c_char_p]
    lib.axon_stop_nrt_profile.restype = ctypes.c_int64

    @contextlib.contextmanager
    def _hook(output_dir, device_ids):
        import jax

        jax.devices()
        if device_ids:
            ids = (ctypes.c_int64 * len(device_ids))(*device_ids)
            rc = lib.axon_start_nrt_profile(ids, len(device_ids))
        else:
            rc = lib.axon_start_nrt_profile(None, 0)
        if rc != 0:
            raise RuntimeError(f"axon_start_nrt_profile rc={rc}")
        try:
            yield
        finally:
            n = lib.axon_stop_nrt_profile(str(output_dir).encode())
            print(f"ntff profile: {n} file(s) written to {output_dir}")

    mod = types.ModuleType("antenv.axon_hooks")
    _state = {"hook": _hook}
    mod.set_axon_ntff_profile_hook = lambda h: _state.__setitem__("hook", h)
    mod.get_axon_ntff_profile_hook = lambda: _state["hook"]
    sys.modules["antenv.axon_hooks"] = mod
    bass_utils.upload_artifacts = lambda tmpdir: tmpdir


_NC_CACHE = {}


def kernel(enc_outputs: np.ndarray, dec_outputs: np.ndarray) -> np.ndarray:
    B, seq, d = dec_outputs.shape
    assert enc_outputs.shape == (B, seq, d) and B == N_CORES

    trace = os.environ.get("BASSKERNEL_TRACE", "0") == "1"
    if trace:
        _install_profile_hook()

    key = (seq, d)
    if key not in _NC_CACHE:
        _NC_CACHE[key] = build(seq, d)
    nc = _NC_CACHE[key]

    in_maps = [
        {
            "dec": np.ascontiguousarray(dec_outputs[b], dtype=np.float32),
            "enc": np.ascontiguousarray(enc_outputs[b], dtype=np.float32),
        }
        for b in range(B)
    ]
    res = bass_utils.run_bass_kernel_spmd(
        nc,
        in_maps,
        core_ids=list(range(N_CORES)),
        trace=trace,
        tmpdir=os.environ.get("BASSKERNEL_TRACE_DIR") if trace else None,
    )
    global LAST_EXEC_TIME_NS
    LAST_EXEC_TIME_NS = res.exec_time_ns
    out = np.stack([res.results[b]["out"] for b in range(B)], axis=0)
    return out.astype(np.float32)


# revision 19
# speedup vs baseline: 1.1730x; 1.1730x over previous
"""Trainium2 Bass kernel for batched cross-attention (CoupletsAttentionModel).

Reference computation (per batch element b):
    S = dec @ enc^T          [S_dec, S_enc]
    P = softmax(S, axis=-1)
    O = P @ enc              [S_dec, D]

Sharding: data-parallel over batch — B=8 batch elements, one per NeuronCore.
Each core runs an identical (SPMD) program on its own batch slice; no
collectives, host stacks the 8 per-core outputs.

Per-core algorithm (S_enc=S_dec=2048, D=512, fp32 in/out), v4:
  - fp16 matmuls (4x faster than fp32; out rel err ~2e-3, tolerance 2e-2).
  - Streamed prologue: dec tiles 0-2 (gpsimd queue) + enc tiles (sync/scalar
    queues, alternating) load while casts (f32->f16), PE transposes
    (enc^T/dec^T), and q-tile 0's S matmuls run incrementally per arriving
    enc chunk — prep is DMA-bound instead of serialized before compute.
  - Software-pipelined main loop, per q-tile iteration:
      mm2(qt-1)  [PE ready immediately — its P^T transpose ran last iter]
      mm1(qt+1)  [fills score PSUM banks as exp(qt) releases them]
      softmax(qt): 4x chunk max (DVE) -> global max -> single exp pass
                   (ScalarE, accum row-sums) — no chunk-rescale multiplies
      P^T via one xbar transpose-DMA [128,2048]->[128,16,128] on SP queue
      dec tile qt+3 prep (gpsimd DMA + cast + PE transpose)
  - PSUM budget: 5 score banks [128,512]f32 + 2 out banks + 1 transpose = 8.
  - Engine split: SP = transposes + half enc loads; Scalar = exp + casts +
    out stores + half enc loads; GpSimd = dec loads; DVE = reductions/scales.
"""

import contextlib
import ctypes
import os
import sys
import types

import numpy as np

import concourse.bass as bass
import concourse.tile as tile
from concourse import bacc, mybir
from concourse import bass_utils
from concourse.masks import make_identity

F32 = mybir.dt.float32
F16 = mybir.dt.float16
AX = mybir.AxisListType
AFT = mybir.ActivationFunctionType

N_CORES = 8
PART = 128


def attention_tile_kernel(tc, out_ap, dec_ap, enc_ap, seq, d):
    nc = tc.nc
    P = PART
    KC = 512  # score chunk width = one fp32 PSUM bank
    n_qt = seq // P
    n_kt = seq // P
    n_dt = d // P
    n_ch = seq // KC
    kt_per_ch = KC // P

    stack = contextlib.ExitStack()
    pool = lambda **kw: stack.enter_context(tc.tile_pool(**kw))

    singles = pool(name="singles", bufs=1)
    big = pool(name="big", bufs=1)
    stage = pool(name="stage", bufs=3)
    stage16 = pool(name="stage16", bufs=3)
    encg_pool = pool(name="encg", bufs=4)
    psum = pool(name="psum", bufs=1, space="PSUM")
    p_pool = pool(name="p_pool", bufs=3)
    pt_pool = pool(name="pt_pool", bufs=2)
    stats = pool(name="stats", bufs=4)
    osb = pool(name="osb", bufs=2)

    with stack:
        # dec tile 0 DMA first on the scalar queue so its data is in flight
        # during engine startup (gpsimd is busy with make_identity).
        d32_0 = stage.tile([P, d], F32, tag="ld32", name="d32_0")
        nc.scalar.dma_start(out=d32_0[:], in_=dec_ap[0:P, :])

        ident = singles.tile([P, P], F16)
        make_identity(nc, ident[:])

        d32_12 = singles.tile([P, 2, d], F32, name="d32_12")
        nc.scalar.dma_start(
            out=d32_12[:],
            in_=dec_ap[P : 3 * P, :].rearrange("(i p) c -> p i c", p=P),
        )

        v_sb = big.tile([P, n_kt, d], F16)  # enc natural (V)
        kT = big.tile([P, n_dt, seq], F16)  # enc^T  [d_in, dt, k]
        qT = big.tile([P, n_dt, seq], F16)  # dec^T  [d_in, dt, q]

        # enc as 4 grouped DMAs (one trigger per 1MB: per-tile triggers leave
        # ~1us of dead time between 0.7us transfers), alternating sync/scalar
        # trigger queues so descriptor generation overlaps transfer.
        eb = []
        for g in range(n_ch):
            ebg = encg_pool.tile([P, kt_per_ch, d], F32, tag="encg", name=f"eb_{g}")
            eng = nc.sync if g % 2 == 0 else nc.scalar
            eng.dma_start(
                out=ebg[:],
                in_=enc_ap[g * KC : (g + 1) * KC, :].rearrange(
                    "(i p) c -> p i c", p=P
                ),
            )
            eb.append(ebg)

        # HAM warmup: dense burst of dummy matmuls during the DMA wait keeps
        # the PE clock ramping (2.4 GHz needs ~3us of continuous activity;
        # idle drops it to 1.2 GHz). Sourced from ident; output never read.
        warm_ps = psum.tile([P, d], F32, tag="tps", bufs=2, name="warm_ps")
        for i in range(44):
            nc.tensor.matmul(
                warm_ps[:, :P], ident[:], ident[:], start=(i == 0), stop=(i == 43)
            )

        def transpose4(dst4, src16, base):
            # 4 PE transposes into one PSUM bank, one [128, 4, 128] copy out
            tps = psum.tile(
                [P, n_dt, P], F16, tag="tps", bufs=2,
                name=f"tps_{dst4.tensor.name}_{base}",
            )
            for j in range(n_dt):
                nc.tensor.transpose(
                    tps[:, j, :], src16[:, (base + j) * P : (base + j + 1) * P],
                    ident[:],
                )
            nc.vector.tensor_copy(dst4[:], tps[:])

        def prep_transposed(src16, dstT, st):
            transpose4(dstT[:, :, st * P : (st + 1) * P], src16, 0)

        def prep_dec_from(src32, j):
            d16 = stage16.tile([P, d], F16, tag="d16", name=f"d16_{j}")
            nc.vector.tensor_copy(d16[:], src32)
            prep_transposed(d16[:], qT, j)

        def prep_dec(j):
            d32 = stage.tile([P, d], F32, tag="ld32", name=f"d32_{j}")
            nc.sync.dma_start(out=d32[:], in_=dec_ap[j * P : (j + 1) * P, :])
            prep_dec_from(d32[:], j)

        # ---- phase A: stream enc groups; build kT/v_sb/qT0-2; qt0's S ----
        prep_dec_from(d32_0[:], 0)
        s_cur = [
            psum.tile([P, KC], F32, tag="s_ch", bufs=5, name=f"s_0_{c}")
            for c in range(n_ch)
        ]
        nmx_cur = stats.tile([P, n_ch], F32, tag="nmx", name="nmx_0")
        for g in range(n_ch):
            for j in range(kt_per_ch):
                st = g * kt_per_ch + j
                nc.scalar.activation(v_sb[:, st, :], eb[g][:, j, :], AFT.Copy)
                prep_transposed(v_sb[:, st, :], kT, st)
            for dt in range(n_dt):
                nc.tensor.matmul(
                    s_cur[g][:],
                    qT[:, dt, 0:P],
                    kT[:, dt, g * KC : (g + 1) * KC],
                    start=(dt == 0),
                    stop=(dt == n_dt - 1),
                )
            nc.vector.tensor_reduce(
                nmx_cur[:, g : g + 1], s_cur[g][:], axis=AX.X,
                op=mybir.AluOpType.max, negate=True,
            )
            if g < 2:
                prep_dec_from(d32_12[:, g, :], g + 1)

        pending = None  # (pT3, rinv) of tile awaiting mm2

        def mm2(qt, pT3, rinv):
            o_ch = psum.tile([P, d], F32, tag="o_ch", bufs=1, name=f"o_ch_{qt}")
            for kt in range(n_kt):
                nc.tensor.matmul(
                    o_ch[:],
                    pT3[:, kt, :],
                    v_sb[:, kt, :],
                    start=(kt == 0),
                    stop=(kt == n_kt - 1),
                )
            o_sb = osb.tile([P, d], F32, tag="osb", name=f"o_sb_{qt}")
            nc.vector.tensor_scalar_mul(o_sb[:], o_ch[:], rinv[:])
            nc.sync.dma_start(out=out_ap[qt * P : (qt + 1) * P, :], in_=o_sb[:])

        # ---- software-pipelined main loop ----
        for qt in range(n_qt):
            # softmax(qt): global max -> exp chunks (ScalarE, row-sum accum)
            negmg = stats.tile([P, 1], F32, tag="negmg", name=f"negmg_{qt}")
            nc.vector.tensor_reduce(
                negmg[:], nmx_cur[:], axis=AX.X, op=mybir.AluOpType.min
            )
            p_sb = p_pool.tile([P, seq], F16, tag="p", name=f"p_{qt}")
            sums = stats.tile([P, n_ch], F32, tag="sums", name=f"sums_{qt}")
            for c in range(n_ch):
                nc.scalar.activation(
                    p_sb[:, c * KC : (c + 1) * KC],
                    s_cur[c][:],
                    AFT.Exp,
                    bias=negmg[:],
                    scale=1.0,
                    accum_out=sums[:, c : c + 1],
                )
            sm = stats.tile([P, 1], F32, tag="sm", name=f"sm_{qt}")
            nc.vector.reduce_sum(sm[:], sums[:], axis=AX.X)
            rinv = stats.tile([P, 1], F32, tag="rinv", name=f"rinv_{qt}")
            nc.vector.reciprocal(rinv[:], sm[:])

            # mm2 of the previous tile first: the PE starts immediately on it
            # while exp(qt) runs and frees score banks.
            if pending is not None:
                mm2(qt - 1, *pending)

            # P^T on the PE (16 transposes, ~0.9us): the xbar transpose-DMA
            # takes ~6.3us serialized (1.7us fixed trigger + 4.5us transfer)
            # which exceeds the 6.8us PE iteration and stalls mm2 every tile.
            pT3 = pt_pool.tile([P, n_kt, P], F16, tag="pT", name=f"pT_{qt}")
            for c in range(n_ch):
                transpose4(
                    pT3[:, c * kt_per_ch : (c + 1) * kt_per_ch, :],
                    p_sb, c * kt_per_ch,
                )
            pending = (pT3, rinv)

            # mm1(qt+1), each chunk's row-max reduced right behind it
            if qt + 1 < n_qt:
                q0 = (qt + 1) * P
                s_nxt = [
                    psum.tile([P, KC], F32, tag="s_ch", bufs=5, name=f"s_{qt+1}_{c}")
                    for c in range(n_ch)
                ]
                nmx_nxt = stats.tile([P, n_ch], F32, tag="nmx", name=f"nmx_{qt+1}")
                for c in range(n_ch):
                    for dt in range(n_dt):
                        nc.tensor.matmul(
                            s_nxt[c][:],
                            qT[:, dt, q0 : q0 + P],
                            kT[:, dt, c * KC : (c + 1) * KC],
                            start=(dt == 0),
                            stop=(dt == n_dt - 1),
                        )
                    nc.vector.tensor_reduce(
                        nmx_nxt[:, c : c + 1], s_nxt[c][:], axis=AX.X,
                        op=mybir.AluOpType.max, negate=True,
                    )
                s_cur = s_nxt
                nmx_cur = nmx_nxt

            if qt + 3 < n_qt:
                prep_dec(qt + 3)

        mm2(n_qt - 1, *pending)


def build(seq=2048, d=512, n_cores=N_CORES):
    nc = bacc.Bacc(
        "TRN2", target_bir_lowering=False, debug=False, num_devices=n_cores
    )
    dec = nc.dram_tensor("dec", [seq, d], F32, kind="ExternalInput").ap()
    enc = nc.dram_tensor("enc", [seq, d], F32, kind="ExternalInput").ap()
    out = nc.dram_tensor("out", [seq, d], F32, kind="ExternalOutput").ap()
    with tile.TileContext(nc) as tc:
        attention_tile_kernel(tc, out, dec, enc, seq, d)
    nc.compile()
    return nc


# ---------------------------------------------------------------------------
# Optional NTFF profiling support (used by our own test harness; inert unless
# BASSKERNEL_TRACE=1). The agent image lacks `antenv.axon_hooks`, so recreate
# it in sys.modules with a ctypes hook against libaxon_pjrt.so.
# ---------------------------------------------------------------------------
LAST_EXEC_TIME_NS = None


def _install_profile_hook():
    so_path = "/opt/axon/libaxon_pjrt.so"
    if "antenv.axon_hooks" in sys.modules or not os.path.exists(so_path):
        return
    lib = ctypes.CDLL(so_path)
    if not hasattr(lib, "axon_start_nrt_profile"):
        return
    lib.axon_start_nrt_profile.argtypes = [
        ctypes.POINTER(ctypes.c_int64),
        ctypes.c_size_t,
    ]
    lib.axon_start_nrt_profile.restype = ctypes.c_int64
    lib.axon_stop_nrt_profile.argtypes = [ctypes.c_char_p]
    lib.axon_stop_nrt_profile.restype = ctypes.c_int64

    @contextlib.contextmanager
    def _hook(output_dir, device_ids):
        import jax

        jax.devices()
        if device_ids:
            ids = (ctypes.c_int64 * len(device_ids))(*device_ids)
            rc = lib.axon_start_nrt_profile(ids, len(device_ids))
        else:
            rc = lib.axon_start_nrt_profile(None, 0)
        if rc != 0:
            raise RuntimeError(f"axon_start_nrt_profile rc={rc}")
        try:
            yield
        finally:
            n = lib.axon_stop_nrt_profile(str(output_dir).encode())
            print(f"ntff profile: {n} file(s) written to {output_dir}")

    mod = types.ModuleType("antenv.axon_hooks")
    _state = {"hook": _hook}
    mod.set_axon_ntff_profile_hook = lambda h: _state.__setitem__("hook", h)
    mod.get_axon_ntff_profile_hook = lambda: _state["hook"]
    sys.modules["antenv.axon_hooks"] = mod
    bass_utils.upload_artifacts = lambda tmpdir: tmpdir


_NC_CACHE = {}


def kernel(enc_outputs: np.ndarray, dec_outputs: np.ndarray) -> np.ndarray:
    B, seq, d = dec_outputs.shape
    assert enc_outputs.shape == (B, seq, d) and B == N_CORES

    trace = os.environ.get("BASSKERNEL_TRACE", "0") == "1"
    if trace:
        _install_profile_hook()

    key = (seq, d)
    if key not in _NC_CACHE:
        _NC_CACHE[key] = build(seq, d)
    nc = _NC_CACHE[key]

    in_maps = [
        {
            "dec": np.ascontiguousarray(dec_outputs[b], dtype=np.float32),
            "enc": np.ascontiguousarray(enc_outputs[b], dtype=np.float32),
        }
        for b in range(B)
    ]
    res = bass_utils.run_bass_kernel_spmd(
        nc,
        in_maps,
        core_ids=list(range(N_CORES)),
        trace=trace,
        tmpdir=os.environ.get("BASSKERNEL_TRACE_DIR") if trace else None,
    )
    global LAST_EXEC_TIME_NS
    LAST_EXEC_TIME_NS = res.exec_time_ns
    out = np.stack([res.results[b]["out"] for b in range(B)], axis=0)
    return out.astype(np.float32)
